# revision 5
# baseline (speedup 1.0000x reference)
"""Distributed causal attention (RoPE, QKV/out projections) on 8 TRN2 NeuronCores.

Sharding: tensor-parallel over heads. H=16 heads -> 2 heads per core.
Each core:
  - reads the full (transposed, bf16) activations xT [D, B*S]
  - computes qT/kT for its 2 heads (column-sharded wq/wk), applies RoPE
  - computes v in natural [s, hd] orientation (for the PV matmul lhsT),
    augmented with a ones-column so PV also produces the softmax denominator
  - flash-style causal attention with scores kept transposed [sk, sq] so
    softmax reduction runs on the TensorEngine via the ones-column trick
  - row-sharded output projection -> partial output [B, D, S]
Host sums the 8 partials and transposes back to [B, S, D].
"""

import numpy as np
import ml_dtypes

import concourse.bass as bass
import concourse.mybir as mybir
from concourse import bacc
import concourse.tile as tile
from concourse.bass import ts, ds

B, S, D, H, HD = 2, 2048, 1024, 16, 64
NCORES = 8
HL = H // NCORES            # heads per core = 2
EL = HL * HD                # local e-dims per core = 128
BS = B * S                  # 4096
DCH = D // 128              # 8 contraction chunks
NCHUNK = BS // 512          # 8 projection chunks (both batches)
SQJ = S // 512              # 4 q-chunks per batch
NKT = S // 128              # 16 k-tiles per batch
THETA = 10000.0
BF = mybir.dt.bfloat16
F32 = mybir.dt.float32
EXPFN = mybir.ActivationFunctionType.Exp

_nc_cache = {}


def build_nc(debug=False):
    key = bool(debug)
    if key in _nc_cache:
        return _nc_cache[key]
    nc = bacc.Bacc("TRN2", target_bir_lowering=False, debug=debug, num_devices=NCORES)

    xT_d = nc.dram_tensor("xT", [D, BS], BF, kind="ExternalInput")
    cos_d = nc.dram_tensor("cosT", [128, S], BF, kind="ExternalInput")
    sin_d = nc.dram_tensor("sinT", [128, S], BF, kind="ExternalInput")
    wq_d = nc.dram_tensor("wqT", [D, EL], BF, kind="ExternalInput")
    wk_d = nc.dram_tensor("wkT", [D, EL], BF, kind="ExternalInput")
    wq2_d = nc.dram_tensor("wq2T", [D, EL], BF, kind="ExternalInput")
    wk2_d = nc.dram_tensor("wk2T", [D, EL], BF, kind="ExternalInput")
    wv_d = nc.dram_tensor("wvT", [D, EL], BF, kind="ExternalInput")
    wo_d = nc.dram_tensor("woT", [EL, D], BF, kind="ExternalInput")
    out_d = nc.dram_tensor("out", [B, D, S], BF, kind="ExternalOutput")

    with tile.TileContext(nc) as tc:
        with (
            tc.tile_pool(name="sb", bufs=1) as sb,
            tc.tile_pool(name="work", bufs=2) as work,
            tc.tile_pool(name="ps", bufs=1, space="PSUM") as ps,
        ):
            # ---- persistent SBUF tensors ----
            xts = sb.tile([128, DCH, BS], BF)      # x transposed, d on partitions
            wqs = sb.tile([128, DCH, EL], BF)
            wks = sb.tile([128, DCH, EL], BF)
            wq2s = sb.tile([128, DCH, EL], BF)
            wk2s = sb.tile([128, DCH, EL], BF)
            wvs = sb.tile([128, DCH, EL], BF)
            wos = sb.tile([128, D], BF)
            coss = sb.tile([128, S], BF)
            sins = sb.tile([128, S], BF)
            qt = sb.tile([128, BS], BF)
            kt = sb.tile([128, BS], BF)
            vaug = sb.tile([128, B * NKT, 130], BF)  # per k-tile: [v_h0|1|v_h1|1]
            attnT = sb.tile([128, BS], BF)           # normalized attn out, heads stacked

            # ---- load inputs ----
            for k in range(DCH):
                nc.sync.dma_start(out=xts[:, k, :], in_=xT_d[ts(k, 128), :])
            nc.sync.dma_start(out=wqs[:], in_=wq_d[:, :].rearrange("(k p) e -> p k e", p=128))
            nc.sync.dma_start(out=wks[:], in_=wk_d[:, :].rearrange("(k p) e -> p k e", p=128))
            nc.sync.dma_start(out=wq2s[:], in_=wq2_d[:, :].rearrange("(k p) e -> p k e", p=128))
            nc.sync.dma_start(out=wk2s[:], in_=wk2_d[:, :].rearrange("(k p) e -> p k e", p=128))
            nc.sync.dma_start(out=wvs[:], in_=wv_d[:, :].rearrange("(k p) e -> p k e", p=128))
            nc.sync.dma_start(out=wos[:], in_=wo_d[:, :])
            nc.sync.dma_start(out=coss[:], in_=cos_d[:, :])
            nc.sync.dma_start(out=sins[:], in_=sin_d[:, :])

            # ones columns for the PV denominator rows (full memset also marks
            # the tensor initialized for the simulator's strided-AP reads)
            nc.gpsimd.memset(vaug[:], 1.0)
            # causal staircase mask for diagonal tiles: keep where f >= p
            mask2 = sb.tile([128, 2, 128], BF)
            nc.gpsimd.memset(mask2[:], 1.0)
            nc.gpsimd.affine_select(
                out=mask2[:], in_=mask2[:],
                compare_op=mybir.AluOpType.is_ge, fill=0.0, base=0,
                pattern=[[0, 2], [1, 128]], channel_multiplier=-1,
            )

            # ---- QKV projections (per 512-col chunk of B*S) ----
            for c in range(NCHUNK):
                cs = ds(c * 512, 512)
                scol = ds((c % SQJ) * 512, 512)     # position columns within batch
                for wtile, w2tile, rot in (
                    (wqs, wq2s, qt),
                    (wks, wk2s, kt),
                ):
                    pp = ps.tile([128, 512], F32, tag="big", bufs=2, name=f"pp{c}")
                    for k in range(DCH):
                        nc.tensor.matmul(
                            pp[:], wtile[:, k, :], xts[:, k, cs],
                            start=(k == 0), stop=(k == DCH - 1),
                        )
                    pp2 = ps.tile([128, 512], F32, tag="big", bufs=2, name=f"pp2{c}")
                    for k in range(DCH):
                        nc.tensor.matmul(
                            pp2[:], w2tile[:, k, :], xts[:, k, cs],
                            start=(k == 0), stop=(k == DCH - 1),
                        )
                    # rope: rot = raw*cos + raw2*sin   (raw2 = signed-pair-swapped proj)
                    rtmp = work.tile([128, 512], BF, tag="ropetmp", bufs=2, name="rtmp")
                    nc.vector.tensor_mul(rot[:, cs], pp[:], coss[:, scol])
                    nc.vector.tensor_mul(rtmp[:], pp2[:], sins[:, scol])
                    nc.vector.tensor_add(rot[:, cs], rot[:, cs], rtmp[:])
                # v in natural orientation: out [s=128, e'=128] per s-tile
                for st in range(4):
                    t128 = c * 4 + st
                    vp = ps.tile([128, 128], F32, tag="big", bufs=2, name=f"vp{t128}")
                    for k in range(DCH):
                        nc.tensor.matmul(
                            vp[:], xts[:, k, ds(t128 * 128, 128)], wvs[:, k, :],
                            start=(k == 0), stop=(k == DCH - 1),
                        )
                    dst = vaug[:, t128, :].rearrange("p (g y) -> p g y", g=2)[:, :, 0:64]
                    src = vp[:].rearrange("p (g y) -> p g y", g=2)
                    nc.scalar.copy(dst, src)

            # ---- attention per (batch, 512-wide q-chunk) ----
            for b in range(B):
                for j in range(SQJ):
                    ntk = 4 * (j + 1)
                    pv = ps.tile([65, 1024], F32, tag="pv", bufs=2, name=f"pv{b}{j}")
                    qc0 = b * S + j * 512
                    for t in range(ntk):
                        off = max(0, 128 * (t - 4 * j))
                        w = 512 - off
                        sc = ps.tile([128, 2, 512], F32, tag="big", bufs=2, name="sc")
                        pt = work.tile([128, 2, 512], BF, tag="ptile", bufs=3, name="pt")
                        kc = b * S + t * 128
                        nc.tensor.matmul(
                            sc[:, 0, off:512], kt[0:64, ds(kc, 128)],
                            qt[0:64, ds(qc0 + off, w)], start=True, stop=True,
                        )
                        nc.tensor.matmul(
                            sc[:, 1, off:512], kt[64:128, ds(kc, 128)],
                            qt[64:128, ds(qc0 + off, w)], start=True, stop=True,
                        )
                        nc.scalar.activation(
                            pt[:, :, off:512], sc[:, :, off:512], EXPFN, scale=0.125,
                        )
                        if t >= 4 * j:  # diagonal: zero the sub-diagonal staircase
                            nc.vector.tensor_mul(
                                pt[:, :, off:off + 128], pt[:, :, off:off + 128],
                                mask2[:],
                            )
                        bt = b * NKT + t
                        nc.tensor.matmul(
                            pv[:, ds(off, w)], vaug[:, bt, 0:65], pt[:, 0, off:512],
                            start=(t == 0), stop=(t == ntk - 1),
                        )
                        nc.tensor.matmul(
                            pv[:, ds(512 + off, w)], vaug[:, bt, 65:130], pt[:, 1, off:512],
                            start=(t == 0), stop=(t == ntk - 1),
                        )
                    # normalize by the denominator row (pv row 64)
                    lbuf = work.tile([1, 1024], F32, tag="lbuf", bufs=2, name="lbuf")
                    rbuf = work.tile([1, 1024], F32, tag="rbuf", bufs=2, name="rbuf")
                    rb = work.tile([64, 1024], F32, tag="rb", bufs=2, name="rb")
                    nc.vector.tensor_copy(lbuf[:], pv[64:65, :])
                    nc.vector.reciprocal_approx_fast(rbuf[:], lbuf[:])
                    nc.gpsimd.partition_broadcast(rb[:], rbuf[:], channels=64)
                    oc = ds(b * S + j * 512, 512)
                    nc.vector.tensor_mul(attnT[0:64, oc], pv[0:64, 0:512], rb[:, 0:512])
                    nc.vector.tensor_mul(attnT[64:128, oc], pv[0:64, 512:1024], rb[:, 512:1024])
                    # output projection (row-sharded wo) for this chunk
                    ost = work.tile([128, 8, 512], BF, tag="ostage", bufs=2, name="ost")
                    for e in range(8):
                        op = ps.tile([128, 512], F32, tag="big", bufs=2, name="op")
                        nc.tensor.matmul(
                            op[:], wos[:, ts(e, 128)], attnT[:, oc],
                            start=True, stop=True,
                        )
                        nc.vector.tensor_copy(ost[:, e, :], op[:])
                    nc.sync.dma_start(
                        out=out_d[b].rearrange("(ec p) s -> p ec s", p=128)[:, :, ts(j, 512)],
                        in_=ost[:],
                    )

    nc.compile()
    _nc_cache[key] = nc
    return nc


def make_in_maps(x, token_positions, wq, wk, wv, wo):
    bf = ml_dtypes.bfloat16
    xT = np.ascontiguousarray(
        np.asarray(x, np.float32).transpose(2, 0, 1).reshape(D, BS)
    ).astype(bf)
    pos = np.asarray(token_positions, np.float64)
    inv_freq = THETA ** (-(2.0 * np.arange(HD // 2, dtype=np.float64) / HD))
    ang = pos[:, None] * inv_freq[None, :]          # [S, 32]
    cos = np.cos(ang).astype(np.float32)
    sin = np.sin(ang).astype(np.float32)
    p = np.arange(128)
    idx = (p % HD) // 2
    cosT = np.ascontiguousarray(cos[:, idx].T).astype(bf)             # [128, S]
    sinT = np.ascontiguousarray(sin[:, idx].T).astype(bf)

    wq = np.asarray(wq, np.float32)
    wk = np.asarray(wk, np.float32)
    wv = np.asarray(wv, np.float32)
    wo = np.asarray(wo, np.float32)

    def swap2(w):  # rows: even p -> -w[p+1], odd p -> +w[p-1]
        w2 = np.empty_like(w)
        w2[0::2] = -w[1::2]
        w2[1::2] = w[0::2]
        return w2

    in_maps = []
    for c in range(NCORES):
        rows = slice(c * EL, (c + 1) * EL)
        in_maps.append({
            "xT": xT,
            "cosT": cosT,
            "sinT": sinT,
            "wqT": np.ascontiguousarray(wq[rows, :].T).astype(bf),
            "wkT": np.ascontiguousarray(wk[rows, :].T).astype(bf),
            "wq2T": np.ascontiguousarray(swap2(wq[rows, :]).T).astype(bf),
            "wk2T": np.ascontiguousarray(swap2(wk[rows, :]).T).astype(bf),
            "wvT": np.ascontiguousarray(wv[rows, :].T).astype(bf),
            "woT": np.ascontiguousarray(wo[:, rows].T).astype(bf),
        })
    return in_maps


def unshard(results):
    acc = np.zeros((B, D, S), np.float32)
    for r in results:
        acc += np.asarray(r["out"], np.float32)
    return np.ascontiguousarray(acc.transpose(0, 2, 1))


def kernel(x, token_positions, wq, wk, wv, wo):
    from concourse.bass_utils import run_bass_kernel_spmd

    nc = build_nc(debug=False)
    in_maps = make_in_maps(x, token_positions, wq, wk, wv, wo)
    res = run_bass_kernel_spmd(nc, in_maps, core_ids=list(range(NCORES)))
    return unshard(res.results)


if __name__ == "__main__":
    # smoke test with random data
    rng = np.random.default_rng(0)
    x = rng.standard_normal((B, S, D), dtype=np.float32)
    tp = np.arange(S, dtype=np.int32)
    ws = [rng.standard_normal((D, D), dtype=np.float32) * 0.02 for _ in range(4)]
    out = kernel(x, tp, *ws)
    print(out.shape, out.dtype)


# revision 7
# speedup vs baseline: 1.0249x; 1.0249x over previous
"""Distributed causal attention (RoPE, QKV/out projections) on 8 TRN2 NeuronCores.

Sharding: tensor-parallel over heads. H=16 heads -> 2 heads per core.
Each core:
  - reads the full (transposed, bf16) activations xT [D, B*S]
  - computes qT/kT for its 2 heads (column-sharded wq/wk), applies RoPE
  - computes v in natural [s, hd] orientation (for the PV matmul lhsT),
    augmented with a ones-column so PV also produces the softmax denominator
  - flash-style causal attention with scores kept transposed [sk, sq] so
    softmax reduction runs on the TensorEngine via the ones-column trick
  - row-sharded output projection -> partial output [B, D, S]
Host sums the 8 partials and transposes back to [B, S, D].
"""

import numpy as np
import ml_dtypes

import concourse.bass as bass
import concourse.mybir as mybir
from concourse import bacc
import concourse.tile as tile
from concourse.bass import ts, ds

B, S, D, H, HD = 2, 2048, 1024, 16, 64
NCORES = 8
HL = H // NCORES            # heads per core = 2
EL = HL * HD                # local e-dims per core = 128
BS = B * S                  # 4096
DCH = D // 128              # 8 contraction chunks
NCHUNK = BS // 512          # 8 projection chunks (both batches)
SQJ = S // 512              # 4 q-chunks per batch
NKT = S // 128              # 16 k-tiles per batch
THETA = 10000.0
BF = mybir.dt.bfloat16
F32 = mybir.dt.float32
EXPFN = mybir.ActivationFunctionType.Exp

_nc_cache = {}


def build_nc(debug=False):
    key = bool(debug)
    if key in _nc_cache:
        return _nc_cache[key]
    nc = bacc.Bacc("TRN2", target_bir_lowering=False, debug=debug, num_devices=NCORES)

    xT_d = nc.dram_tensor("xT", [D, BS], BF, kind="ExternalInput")
    cos_d = nc.dram_tensor("cosT", [128, S], BF, kind="ExternalInput")
    sin_d = nc.dram_tensor("sinT", [128, S], BF, kind="ExternalInput")
    wq_d = nc.dram_tensor("wqT", [D, EL], BF, kind="ExternalInput")
    wk_d = nc.dram_tensor("wkT", [D, EL], BF, kind="ExternalInput")
    wq2_d = nc.dram_tensor("wq2T", [D, EL], BF, kind="ExternalInput")
    wk2_d = nc.dram_tensor("wk2T", [D, EL], BF, kind="ExternalInput")
    wv_d = nc.dram_tensor("wvT", [D, EL], BF, kind="ExternalInput")
    wo_d = nc.dram_tensor("woT", [EL, D], BF, kind="ExternalInput")
    out_d = nc.dram_tensor("out", [B, D, S], BF, kind="ExternalOutput")

    with tile.TileContext(nc) as tc:
        with (
            tc.tile_pool(name="sb", bufs=1) as sb,
            tc.tile_pool(name="work", bufs=2) as work,
            tc.tile_pool(name="ps", bufs=1, space="PSUM") as ps,
        ):
            # ---- persistent SBUF tensors ----
            xts = sb.tile([128, DCH, BS], BF)      # x transposed, d on partitions
            wqs = sb.tile([128, DCH, EL], BF)
            wks = sb.tile([128, DCH, EL], BF)
            wq2s = sb.tile([128, DCH, EL], BF)
            wk2s = sb.tile([128, DCH, EL], BF)
            wvs = sb.tile([128, DCH, EL], BF)
            wos = sb.tile([128, D], BF)
            coss = sb.tile([128, S], BF)
            sins = sb.tile([128, S], BF)
            qt = sb.tile([128, BS], BF)
            kt = sb.tile([128, BS], BF)
            vaug = sb.tile([128, B * NKT, 130], BF)  # per k-tile: [v_h0|1|v_h1|1]
            attnT = sb.tile([128, BS], BF)           # normalized attn out, heads stacked

            # ---- load inputs ----
            for k in range(DCH):
                nc.sync.dma_start(out=xts[:, k, :], in_=xT_d[ts(k, 128), :])
            nc.sync.dma_start(out=wqs[:], in_=wq_d[:, :].rearrange("(k p) e -> p k e", p=128))
            nc.sync.dma_start(out=wks[:], in_=wk_d[:, :].rearrange("(k p) e -> p k e", p=128))
            nc.sync.dma_start(out=wq2s[:], in_=wq2_d[:, :].rearrange("(k p) e -> p k e", p=128))
            nc.sync.dma_start(out=wk2s[:], in_=wk2_d[:, :].rearrange("(k p) e -> p k e", p=128))
            nc.sync.dma_start(out=wvs[:], in_=wv_d[:, :].rearrange("(k p) e -> p k e", p=128))
            nc.sync.dma_start(out=wos[:], in_=wo_d[:, :])
            nc.sync.dma_start(out=coss[:], in_=cos_d[:, :])
            nc.sync.dma_start(out=sins[:], in_=sin_d[:, :])

            # ones columns for the PV denominator rows (full memset also marks
            # the tensor initialized for the simulator's strided-AP reads)
            nc.gpsimd.memset(vaug[:], 1.0)
            # causal staircase mask for diagonal tiles: keep where f >= p
            mask2 = sb.tile([128, 2, 128], BF)
            nc.gpsimd.memset(mask2[:], 1.0)
            nc.gpsimd.affine_select(
                out=mask2[:], in_=mask2[:],
                compare_op=mybir.AluOpType.is_ge, fill=0.0, base=0,
                pattern=[[0, 2], [1, 128]], channel_multiplier=-1,
            )

            # ---- QKV projection for one 512-col chunk of B*S ----
            def proj_chunk(c):
                cs = ds(c * 512, 512)
                scol = ds((c % SQJ) * 512, 512)     # position columns within batch
                for wtile, w2tile, rot in (
                    (wqs, wq2s, qt),
                    (wks, wk2s, kt),
                ):
                    pp = ps.tile([128, 512], F32, tag="big", bufs=2, name=f"pp{c}")
                    for k in range(DCH):
                        nc.tensor.matmul(
                            pp[:], wtile[:, k, :], xts[:, k, cs],
                            start=(k == 0), stop=(k == DCH - 1),
                        )
                    pp2 = ps.tile([128, 512], F32, tag="big", bufs=2, name=f"pp2{c}")
                    for k in range(DCH):
                        nc.tensor.matmul(
                            pp2[:], w2tile[:, k, :], xts[:, k, cs],
                            start=(k == 0), stop=(k == DCH - 1),
                        )
                    # rope: rot = raw*cos + raw2*sin (raw2 = signed-pair-swapped proj)
                    rtmp = work.tile([128, 512], BF, tag="ropetmp", bufs=2, name="rtmp")
                    nc.vector.tensor_mul(rot[:, cs], pp[:], coss[:, scol])
                    nc.vector.tensor_mul(rtmp[:], pp2[:], sins[:, scol])
                    nc.vector.tensor_add(rot[:, cs], rot[:, cs], rtmp[:])
                # v in natural orientation: out [s=128, e'=128] per s-tile
                for st in range(4):
                    t128 = c * 4 + st
                    vp = ps.tile([128, 128], F32, tag="big", bufs=2, name=f"vp{t128}")
                    for k in range(DCH):
                        nc.tensor.matmul(
                            vp[:], xts[:, k, ds(t128 * 128, 128)], wvs[:, k, :],
                            start=(k == 0), stop=(k == DCH - 1),
                        )
                    dst = vaug[:, t128, :].rearrange("p (g y) -> p g y", g=2)[:, :, 0:64]
                    src = vp[:].rearrange("p (g y) -> p g y", g=2)
                    nc.scalar.copy(dst, src)

            # ---- causal attention for one (batch, 512-wide q-chunk) ----
            def attn_chunk(b, j):
                ntk = 4 * (j + 1)
                pv = ps.tile([65, 1024], F32, tag="pv", bufs=2, name=f"pv{b}{j}")
                qc0 = b * S + j * 512
                for t in range(ntk):
                    off = max(0, 128 * (t - 4 * j))
                    w = 512 - off
                    sc = ps.tile([128, 2, 512], F32, tag="big", bufs=2, name="sc")
                    pt = work.tile([128, 2, 512], BF, tag="ptile", bufs=3, name="pt")
                    kc = b * S + t * 128
                    nc.tensor.matmul(
                        sc[:, 0, off:512], kt[0:64, ds(kc, 128)],
                        qt[0:64, ds(qc0 + off, w)], start=True, stop=True,
                    )
                    nc.tensor.matmul(
                        sc[:, 1, off:512], kt[64:128, ds(kc, 128)],
                        qt[64:128, ds(qc0 + off, w)], start=True, stop=True,
                    )
                    nc.scalar.activation(
                        pt[:, :, off:512], sc[:, :, off:512], EXPFN, scale=0.125,
                    )
                    if t >= 4 * j:  # diagonal: zero the sub-diagonal staircase
                        nc.vector.tensor_mul(
                            pt[:, :, off:off + 128], pt[:, :, off:off + 128],
                            mask2[:],
                        )
                    bt = b * NKT + t
                    nc.tensor.matmul(
                        pv[:, ds(off, w)], vaug[:, bt, 0:65], pt[:, 0, off:512],
                        start=(t == 0), stop=(t == ntk - 1),
                    )
                    nc.tensor.matmul(
                        pv[:, ds(512 + off, w)], vaug[:, bt, 65:130], pt[:, 1, off:512],
                        start=(t == 0), stop=(t == ntk - 1),
                    )
                # normalize by the denominator row (pv row 64)
                lbuf = work.tile([1, 1024], F32, tag="lbuf", bufs=2, name="lbuf")
                rbuf = work.tile([1, 1024], F32, tag="rbuf", bufs=2, name="rbuf")
                rb = work.tile([64, 1024], F32, tag="rb", bufs=2, name="rb")
                nc.vector.tensor_copy(lbuf[:], pv[64:65, :])
                nc.vector.reciprocal_approx_fast(rbuf[:], lbuf[:])
                nc.gpsimd.partition_broadcast(rb[:], rbuf[:], channels=64)
                oc = ds(b * S + j * 512, 512)
                nc.vector.tensor_mul(attnT[0:64, oc], pv[0:64, 0:512], rb[:, 0:512])
                nc.vector.tensor_mul(attnT[64:128, oc], pv[0:64, 512:1024], rb[:, 512:1024])

            # ---- output projection for one (batch, q-chunk) ----
            def oproj_chunk(b, j):
                oc = ds(b * S + j * 512, 512)
                ost = work.tile([128, 8, 512], BF, tag="ostage", bufs=2, name="ost")
                for e in range(8):
                    op = ps.tile([128, 512], F32, tag="big", bufs=2, name="op")
                    nc.tensor.matmul(
                        op[:], wos[:, ts(e, 128)], attnT[:, oc],
                        start=True, stop=True,
                    )
                    nc.vector.tensor_copy(ost[:, e, :], op[:])
                nc.sync.dma_start(
                    out=out_d[b].rearrange("(ec p) s -> p ec s", p=128)[:, :, ts(j, 512)],
                    in_=ost[:],
                )

            # interleave projection chunks with attention so the ACT-heavy
            # attention phase overlaps the PE-dense projection of later chunks
            for b in range(B):
                for j in range(SQJ):
                    proj_chunk(b * SQJ + j)
                    attn_chunk(b, j)
            for b in range(B):
                for j in range(SQJ):
                    oproj_chunk(b, j)

    nc.compile()
    _nc_cache[key] = nc
    return nc


def make_in_maps(x, token_positions, wq, wk, wv, wo):
    bf = ml_dtypes.bfloat16
    xT = np.ascontiguousarray(
        np.asarray(x, np.float32).transpose(2, 0, 1).reshape(D, BS)
    ).astype(bf)
    pos = np.asarray(token_positions, np.float64)
    inv_freq = THETA ** (-(2.0 * np.arange(HD // 2, dtype=np.float64) / HD))
    ang = pos[:, None] * inv_freq[None, :]          # [S, 32]
    cos = np.cos(ang).astype(np.float32)
    sin = np.sin(ang).astype(np.float32)
    p = np.arange(128)
    idx = (p % HD) // 2
    cosT = np.ascontiguousarray(cos[:, idx].T).astype(bf)             # [128, S]
    sinT = np.ascontiguousarray(sin[:, idx].T).astype(bf)

    wq = np.asarray(wq, np.float32)
    wk = np.asarray(wk, np.float32)
    wv = np.asarray(wv, np.float32)
    wo = np.asarray(wo, np.float32)

    def swap2(w):  # rows: even p -> -w[p+1], odd p -> +w[p-1]
        w2 = np.empty_like(w)
        w2[0::2] = -w[1::2]
        w2[1::2] = w[0::2]
        return w2

    in_maps = []
    for c in range(NCORES):
        rows = slice(c * EL, (c + 1) * EL)
        in_maps.append({
            "xT": xT,
            "cosT": cosT,
            "sinT": sinT,
            "wqT": np.ascontiguousarray(wq[rows, :].T).astype(bf),
            "wkT": np.ascontiguousarray(wk[rows, :].T).astype(bf),
            "wq2T": np.ascontiguousarray(swap2(wq[rows, :]).T).astype(bf),
            "wk2T": np.ascontiguousarray(swap2(wk[rows, :]).T).astype(bf),
            "wvT": np.ascontiguousarray(wv[rows, :].T).astype(bf),
            "woT": np.ascontiguousarray(wo[:, rows].T).astype(bf),
        })
    return in_maps


def unshard(results):
    acc = np.zeros((B, D, S), np.float32)
    for r in results:
        acc += np.asarray(r["out"], np.float32)
    return np.ascontiguousarray(acc.transpose(0, 2, 1))


def kernel(x, token_positions, wq, wk, wv, wo):
    from concourse.bass_utils import run_bass_kernel_spmd

    nc = build_nc(debug=False)
    in_maps = make_in_maps(x, token_positions, wq, wk, wv, wo)
    res = run_bass_kernel_spmd(nc, in_maps, core_ids=list(range(NCORES)))
    return unshard(res.results)


if __name__ == "__main__":
    # smoke test with random data
    rng = np.random.default_rng(0)
    x = rng.standard_normal((B, S, D), dtype=np.float32)
    tp = np.arange(S, dtype=np.int32)
    ws = [rng.standard_normal((D, D), dtype=np.float32) * 0.02 for _ in range(4)]
    out = kernel(x, tp, *ws)
    print(out.shape, out.dtype)


# revision 8
# speedup vs baseline: 1.2555x; 1.2250x over previous
"""Distributed causal attention (RoPE, QKV/out projections) on 8 TRN2 NeuronCores.

Sharding: tensor-parallel over heads. H=16 heads -> 2 heads per core.
Each core:
  - reads the full (transposed, bf16) activations xT [D, B*S]
  - computes qT/kT for its 2 heads (column-sharded wq/wk), applies RoPE
  - computes v in natural [s, hd] orientation (for the PV matmul lhsT),
    augmented with a ones-column so PV also produces the softmax denominator
  - flash-style causal attention with scores kept transposed [sk, sq] so
    softmax reduction runs on the TensorEngine via the ones-column trick
  - row-sharded output projection -> partial output [B, D, S]
Host sums the 8 partials and transposes back to [B, S, D].
"""

import numpy as np
import ml_dtypes

import concourse.bass as bass
import concourse.mybir as mybir
from concourse import bacc
import concourse.tile as tile
from concourse.bass import ts, ds

B, S, D, H, HD = 2, 2048, 1024, 16, 64
NCORES = 8
HL = H // NCORES            # heads per core = 2
EL = HL * HD                # local e-dims per core = 128
BS = B * S                  # 4096
DCH = D // 128              # 8 contraction chunks
NCHUNK = BS // 512          # 8 projection chunks (both batches)
SQJ = S // 512              # 4 q-chunks per batch
NKT = S // 128              # 16 k-tiles per batch
THETA = 10000.0
BF = mybir.dt.bfloat16
F32 = mybir.dt.float32
EXPFN = mybir.ActivationFunctionType.Exp

_nc_cache = {}


def build_nc(debug=False):
    key = bool(debug)
    if key in _nc_cache:
        return _nc_cache[key]
    nc = bacc.Bacc("TRN2", target_bir_lowering=False, debug=debug, num_devices=NCORES)

    xT_d = nc.dram_tensor("xT", [D, BS], BF, kind="ExternalInput")
    cos_d = nc.dram_tensor("cosT", [128, S], BF, kind="ExternalInput")
    sin_d = nc.dram_tensor("sinT", [128, S], BF, kind="ExternalInput")
    wq_d = nc.dram_tensor("wqT", [D, EL], BF, kind="ExternalInput")
    wk_d = nc.dram_tensor("wkT", [D, EL], BF, kind="ExternalInput")
    wq2_d = nc.dram_tensor("wq2T", [D, EL], BF, kind="ExternalInput")
    wk2_d = nc.dram_tensor("wk2T", [D, EL], BF, kind="ExternalInput")
    wv_d = nc.dram_tensor("wvT", [D, EL], BF, kind="ExternalInput")
    wo_d = nc.dram_tensor("woT", [EL, D], BF, kind="ExternalInput")
    out_d = nc.dram_tensor("out", [B, D, S], BF, kind="ExternalOutput")

    with tile.TileContext(nc) as tc:
        with (
            tc.tile_pool(name="sb", bufs=1) as sb,
            tc.tile_pool(name="work", bufs=2) as work,
            tc.tile_pool(name="ps", bufs=1, space="PSUM") as ps,
        ):
            # ---- persistent SBUF tensors ----
            xts = sb.tile([128, DCH, BS], BF)      # x transposed, d on partitions
            wqs = sb.tile([128, DCH, EL], BF)
            wks = sb.tile([128, DCH, EL], BF)
            wq2s = sb.tile([128, DCH, EL], BF)
            wk2s = sb.tile([128, DCH, EL], BF)
            wvs = sb.tile([128, DCH, EL], BF)
            wos = sb.tile([128, D], BF)
            coss = sb.tile([128, S], BF)
            sins = sb.tile([128, S], BF)
            qt = sb.tile([128, BS], BF)
            kt = sb.tile([128, BS], BF)
            vaug = sb.tile([128, B * NKT, 130], BF)  # per k-tile: [v_h0|1|v_h1|1]
            attnT = sb.tile([128, BS], BF)           # normalized attn out, heads stacked

            # ---- load inputs (x split by column group so compute starts early) ----
            for cg in range(NCHUNK):
                for k in range(DCH):
                    nc.sync.dma_start(
                        out=xts[:, k, ts(cg, 512)], in_=xT_d[ts(k, 128), ts(cg, 512)]
                    )
            nc.sync.dma_start(out=wqs[:], in_=wq_d[:, :].rearrange("(k p) e -> p k e", p=128))
            nc.sync.dma_start(out=wks[:], in_=wk_d[:, :].rearrange("(k p) e -> p k e", p=128))
            nc.sync.dma_start(out=wq2s[:], in_=wq2_d[:, :].rearrange("(k p) e -> p k e", p=128))
            nc.sync.dma_start(out=wk2s[:], in_=wk2_d[:, :].rearrange("(k p) e -> p k e", p=128))
            nc.sync.dma_start(out=wvs[:], in_=wv_d[:, :].rearrange("(k p) e -> p k e", p=128))
            nc.sync.dma_start(out=wos[:], in_=wo_d[:, :])
            nc.sync.dma_start(out=coss[:], in_=cos_d[:, :])
            nc.sync.dma_start(out=sins[:], in_=sin_d[:, :])

            # ones columns for the PV denominator rows (full memset also marks
            # the tensor initialized for the simulator's strided-AP reads)
            nc.gpsimd.memset(vaug[:], 1.0)
            # causal staircase mask for diagonal tiles: keep where f >= p
            scratch1 = sb.tile([128, 1], F32)
            nc.vector.memset(scratch1[:], 0.0)
            nc.scalar.activation(scratch1[:], scratch1[:], EXPFN)  # preload exp table
            mask2 = sb.tile([128, 2, 128], BF)
            nc.gpsimd.memset(mask2[:], 1.0)
            nc.gpsimd.affine_select(
                out=mask2[:], in_=mask2[:],
                compare_op=mybir.AluOpType.is_ge, fill=0.0, base=0,
                pattern=[[0, 2], [1, 128]], channel_multiplier=-1,
            )

            # ---- QKV projection for one 512-col chunk of B*S ----
            def proj_chunk(c):
                cs = ds(c * 512, 512)
                scol = ds((c % SQJ) * 512, 512)     # position columns within batch
                for wtile, w2tile, rot in (
                    (wqs, wq2s, qt),
                    (wks, wk2s, kt),
                ):
                    pp = ps.tile([128, 512], F32, tag="big", bufs=2, name=f"pp{c}")
                    for k in range(DCH):
                        nc.tensor.matmul(
                            pp[:], wtile[:, k, :], xts[:, k, cs],
                            start=(k == 0), stop=(k == DCH - 1),
                        )
                    pp2 = ps.tile([128, 512], F32, tag="big", bufs=2, name=f"pp2{c}")
                    for k in range(DCH):
                        nc.tensor.matmul(
                            pp2[:], w2tile[:, k, :], xts[:, k, cs],
                            start=(k == 0), stop=(k == DCH - 1),
                        )
                    # rope: rot = raw*cos + raw2*sin (raw2 = signed-pair-swapped proj)
                    rtmp = work.tile([128, 512], BF, tag="ropetmp", bufs=2, name="rtmp")
                    nc.vector.tensor_mul(rot[:, cs], pp[:], coss[:, scol])
                    nc.vector.tensor_mul(rtmp[:], pp2[:], sins[:, scol])
                    nc.vector.tensor_add(rot[:, cs], rot[:, cs], rtmp[:])
                # v in natural orientation: out [s=128, e'=128] per s-tile
                for st in range(4):
                    t128 = c * 4 + st
                    vp = ps.tile([128, 128], F32, tag="big", bufs=2, name=f"vp{t128}")
                    for k in range(DCH):
                        nc.tensor.matmul(
                            vp[:], xts[:, k, ds(t128 * 128, 128)], wvs[:, k, :],
                            start=(k == 0), stop=(k == DCH - 1),
                        )
                    dst = vaug[:, t128, :].rearrange("p (g y) -> p g y", g=2)[:, :, 0:64]
                    src = vp[:].rearrange("p (g y) -> p g y", g=2)
                    nc.scalar.copy(dst, src)

            # ---- causal attention for one (batch, 512-wide q-chunk) ----
            def attn_chunk(b, j):
                ntk = 4 * (j + 1)
                pv = ps.tile([65, 1024], F32, tag="pv", bufs=1, name=f"pv{b}{j}")
                qc0 = b * S + j * 512
                for t in range(ntk):
                    off = max(0, 128 * (t - 4 * j))
                    w = 512 - off
                    sc = ps.tile([128, 2, 512], F32, tag="big", bufs=2, name="sc")
                    pt = work.tile([128, 2, 512], BF, tag="ptile", bufs=3, name="pt")
                    kc = b * S + t * 128
                    nc.tensor.matmul(
                        sc[:, 0, off:512], kt[0:64, ds(kc, 128)],
                        qt[0:64, ds(qc0 + off, w)], start=True, stop=True,
                    )
                    nc.tensor.matmul(
                        sc[:, 1, off:512], kt[64:128, ds(kc, 128)],
                        qt[64:128, ds(qc0 + off, w)], start=True, stop=True,
                    )
                    nc.scalar.activation(
                        pt[:, :, off:512], sc[:, :, off:512], EXPFN, scale=0.125,
                    )
                    if t >= 4 * j:  # diagonal: zero the sub-diagonal staircase
                        nc.vector.tensor_mul(
                            pt[:, :, off:off + 128], pt[:, :, off:off + 128],
                            mask2[:],
                        )
                    bt = b * NKT + t
                    nc.tensor.matmul(
                        pv[:, ds(off, w)], vaug[:, bt, 0:65], pt[:, 0, off:512],
                        start=(t == 0), stop=(t == ntk - 1),
                    )
                    nc.tensor.matmul(
                        pv[:, ds(512 + off, w)], vaug[:, bt, 65:130], pt[:, 1, off:512],
                        start=(t == 0), stop=(t == ntk - 1),
                    )
                # normalize by the denominator row (pv row 64)
                lbuf = work.tile([1, 1024], F32, tag="lbuf", bufs=2, name="lbuf")
                rbuf = work.tile([1, 1024], F32, tag="rbuf", bufs=2, name="rbuf")
                rb = work.tile([64, 1024], F32, tag="rb", bufs=2, name="rb")
                nc.vector.tensor_copy(lbuf[:], pv[64:65, :])
                nc.vector.reciprocal_approx_fast(rbuf[:], lbuf[:])
                nc.gpsimd.partition_broadcast(rb[:], rbuf[:], channels=64)
                oc = ds(b * S + j * 512, 512)
                nc.vector.tensor_mul(attnT[0:64, oc], pv[0:64, 0:512], rb[:, 0:512])
                nc.vector.tensor_mul(attnT[64:128, oc], pv[0:64, 512:1024], rb[:, 512:1024])

            # ---- output projection for one (batch, q-chunk) ----
            def oproj_chunk(b, j):
                oc = ds(b * S + j * 512, 512)
                ost = work.tile([128, 8, 512], BF, tag="ostage", bufs=2, name="ost")
                for e in range(8):
                    op = ps.tile([128, 512], F32, tag="op", bufs=2, name="op")
                    nc.tensor.matmul(
                        op[:], wos[:, ts(e, 128)], attnT[:, oc],
                        start=True, stop=True,
                    )
                    if e % 2 == 0:
                        nc.vector.tensor_copy(ost[:, e, :], op[:])
                    else:
                        nc.scalar.copy(ost[:, e, :], op[:])
                nc.sync.dma_start(
                    out=out_d[b].rearrange("(ec p) s -> p ec s", p=128)[:, :, ts(j, 512)],
                    in_=ost[:],
                )

            # interleave: projection of chunk i, attention on chunk i, and the
            # output projection of chunk i-1 (whose normalize finished during
            # attention of chunk i) are adjacent in program order so the PE
            # always has dense work while ACT/DVE chew the previous chunk
            chunks = [(b, j) for b in range(B) for j in range(SQJ)]
            for i, (b, j) in enumerate(chunks):
                proj_chunk(b * SQJ + j)
                attn_chunk(b, j)
                if i > 0:
                    oproj_chunk(*chunks[i - 1])
            oproj_chunk(*chunks[-1])

    nc.compile()
    _nc_cache[key] = nc
    return nc


def make_in_maps(x, token_positions, wq, wk, wv, wo):
    bf = ml_dtypes.bfloat16
    xT = np.ascontiguousarray(
        np.asarray(x, np.float32).transpose(2, 0, 1).reshape(D, BS)
    ).astype(bf)
    pos = np.asarray(token_positions, np.float64)
    inv_freq = THETA ** (-(2.0 * np.arange(HD // 2, dtype=np.float64) / HD))
    ang = pos[:, None] * inv_freq[None, :]          # [S, 32]
    cos = np.cos(ang).astype(np.float32)
    sin = np.sin(ang).astype(np.float32)
    p = np.arange(128)
    idx = (p % HD) // 2
    cosT = np.ascontiguousarray(cos[:, idx].T).astype(bf)             # [128, S]
    sinT = np.ascontiguousarray(sin[:, idx].T).astype(bf)

    wq = np.asarray(wq, np.float32)
    wk = np.asarray(wk, np.float32)
    wv = np.asarray(wv, np.float32)
    wo = np.asarray(wo, np.float32)

    def swap2(w):  # rows: even p -> -w[p+1], odd p -> +w[p-1]
        w2 = np.empty_like(w)
        w2[0::2] = -w[1::2]
        w2[1::2] = w[0::2]
        return w2

    in_maps = []
    for c in range(NCORES):
        rows = slice(c * EL, (c + 1) * EL)
        in_maps.append({
            "xT": xT,
            "cosT": cosT,
            "sinT": sinT,
            "wqT": np.ascontiguousarray(wq[rows, :].T).astype(bf),
            "wkT": np.ascontiguousarray(wk[rows, :].T).astype(bf),
            "wq2T": np.ascontiguousarray(swap2(wq[rows, :]).T).astype(bf),
            "wk2T": np.ascontiguousarray(swap2(wk[rows, :]).T).astype(bf),
            "wvT": np.ascontiguousarray(wv[rows, :].T).astype(bf),
            "woT": np.ascontiguousarray(wo[:, rows].T).astype(bf),
        })
    return in_maps


def unshard(results):
    acc = np.zeros((B, D, S), np.float32)
    for r in results:
        acc += np.asarray(r["out"], np.float32)
    return np.ascontiguousarray(acc.transpose(0, 2, 1))


def kernel(x, token_positions, wq, wk, wv, wo):
    from concourse.bass_utils import run_bass_kernel_spmd

    nc = build_nc(debug=False)
    in_maps = make_in_maps(x, token_positions, wq, wk, wv, wo)
    res = run_bass_kernel_spmd(nc, in_maps, core_ids=list(range(NCORES)))
    return unshard(res.results)


if __name__ == "__main__":
    # smoke test with random data
    rng = np.random.default_rng(0)
    x = rng.standard_normal((B, S, D), dtype=np.float32)
    tp = np.arange(S, dtype=np.int32)
    ws = [rng.standard_normal((D, D), dtype=np.float32) * 0.02 for _ in range(4)]
    out = kernel(x, tp, *ws)
    print(out.shape, out.dtype)


# revision 9
# speedup vs baseline: 1.4791x; 1.1782x over previous
"""Distributed causal attention (RoPE, QKV/out projections) on 8 TRN2 NeuronCores.

Sharding: tensor-parallel over heads. H=16 heads -> 2 heads per core.
Each core:
  - reads the full (transposed, bf16) activations xT [D, B*S]
  - computes qT/kT for its 2 heads (column-sharded wq/wk), applies RoPE
  - computes v in natural [s, hd] orientation (for the PV matmul lhsT),
    augmented with a ones-column so PV also produces the softmax denominator
  - flash-style causal attention with scores kept transposed [sk, sq] so
    softmax reduction runs on the TensorEngine via the ones-column trick
  - row-sharded output projection -> partial output [B, D, S]
Host sums the 8 partials and transposes back to [B, S, D].
"""

import numpy as np
import ml_dtypes

import concourse.bass as bass
import concourse.mybir as mybir
from concourse import bacc
import concourse.tile as tile
from concourse.bass import ts, ds

B, S, D, H, HD = 2, 2048, 1024, 16, 64
NCORES = 8
HL = H // NCORES            # heads per core = 2
EL = HL * HD                # local e-dims per core = 128
BS = B * S                  # 4096
DCH = D // 128              # 8 contraction chunks
NCHUNK = BS // 512          # 8 projection chunks (both batches)
SQJ = S // 512              # 4 q-chunks per batch
NKT = S // 128              # 16 k-tiles per batch
THETA = 10000.0
BF = mybir.dt.bfloat16
F32 = mybir.dt.float32
EXPFN = mybir.ActivationFunctionType.Exp

_nc_cache = {}


def build_nc(debug=False):
    key = bool(debug)
    if key in _nc_cache:
        return _nc_cache[key]
    nc = bacc.Bacc("TRN2", target_bir_lowering=False, debug=debug, num_devices=NCORES)

    xT_d = nc.dram_tensor("xT", [D, BS], BF, kind="ExternalInput")
    cos_d = nc.dram_tensor("cosT", [128, S], BF, kind="ExternalInput")
    sin_d = nc.dram_tensor("sinT", [128, S], BF, kind="ExternalInput")
    wq_d = nc.dram_tensor("wqT", [D, EL], BF, kind="ExternalInput")
    wk_d = nc.dram_tensor("wkT", [D, EL], BF, kind="ExternalInput")
    wq2_d = nc.dram_tensor("wq2T", [D, EL], BF, kind="ExternalInput")
    wk2_d = nc.dram_tensor("wk2T", [D, EL], BF, kind="ExternalInput")
    wv_d = nc.dram_tensor("wvT", [D, EL], BF, kind="ExternalInput")
    wo_d = nc.dram_tensor("woT", [EL, D], BF, kind="ExternalInput")
    out_d = nc.dram_tensor("out", [B, D, S], BF, kind="ExternalOutput")

    with tile.TileContext(nc) as tc:
        with (
            tc.tile_pool(name="sb", bufs=1) as sb,
            tc.tile_pool(name="work", bufs=2) as work,
            tc.tile_pool(name="ps", bufs=1, space="PSUM") as ps,
        ):
            # ---- persistent SBUF tensors ----
            xts = sb.tile([128, DCH, BS], BF)      # x transposed, d on partitions
            wqs = sb.tile([128, DCH, EL], BF)
            wks = sb.tile([128, DCH, EL], BF)
            wq2s = sb.tile([128, DCH, EL], BF)
            wk2s = sb.tile([128, DCH, EL], BF)
            wvs = sb.tile([128, DCH, EL], BF)
            wos = sb.tile([128, D], BF)
            coss = sb.tile([128, S], BF)
            sins = sb.tile([128, S], BF)
            qt = sb.tile([128, BS], BF)
            kt = sb.tile([128, BS], BF)
            vaug = sb.tile([128, B * NKT, 130], BF)  # per k-tile: [v_h0|1|v_h1|1]
            attnT = sb.tile([128, BS], BF)           # normalized attn out, heads stacked

            # ---- load inputs: tiny weight/table DMAs first, then x by column
            # group (first group small so the first projection starts early) ----
            nc.sync.dma_start(out=wqs[:], in_=wq_d[:, :].rearrange("(k p) e -> p k e", p=128))
            nc.sync.dma_start(out=wq2s[:], in_=wq2_d[:, :].rearrange("(k p) e -> p k e", p=128))
            nc.sync.dma_start(out=wks[:], in_=wk_d[:, :].rearrange("(k p) e -> p k e", p=128))
            nc.sync.dma_start(out=wk2s[:], in_=wk2_d[:, :].rearrange("(k p) e -> p k e", p=128))
            nc.sync.dma_start(out=wvs[:], in_=wv_d[:, :].rearrange("(k p) e -> p k e", p=128))
            nc.sync.dma_start(out=coss[:], in_=cos_d[:, :])
            nc.sync.dma_start(out=sins[:], in_=sin_d[:, :])
            nc.sync.dma_start(out=wos[:], in_=wo_d[:, :])
            for k in range(DCH):
                nc.sync.dma_start(
                    out=xts[:, k, ts(0, 512)], in_=xT_d[ts(k, 128), ts(0, 512)]
                )
            for cg in range(1, 4):
                for k in range(DCH):
                    nc.sync.dma_start(
                        out=xts[:, k, ds(cg * 1024 - 512, 1024)],
                        in_=xT_d[ts(k, 128), ds(cg * 1024 - 512, 1024)],
                    )
            for k in range(DCH):
                nc.sync.dma_start(
                    out=xts[:, k, ds(3584, 512)], in_=xT_d[ts(k, 128), ds(3584, 512)]
                )

            # ones columns for the PV denominator rows (full memset also marks
            # the tensor initialized for the simulator's strided-AP reads)
            nc.gpsimd.memset(vaug[:], 1.0)
            # causal staircase mask for diagonal tiles: keep where f >= p
            scratch1 = sb.tile([128, 1], F32)
            nc.vector.memset(scratch1[:], 0.0)
            nc.scalar.activation(scratch1[:], scratch1[:], EXPFN)  # preload exp table
            mask2 = sb.tile([128, 2, 128], BF)
            nc.gpsimd.memset(mask2[:], 1.0)
            nc.gpsimd.affine_select(
                out=mask2[:], in_=mask2[:],
                compare_op=mybir.AluOpType.is_ge, fill=0.0, base=0,
                pattern=[[0, 2], [1, 128]], channel_multiplier=-1,
            )

            # ---- QKV projection for one 512-col chunk of B*S ----
            def proj_chunk(c):
                cs = ds(c * 512, 512)
                scol = ds((c % SQJ) * 512, 512)     # position columns within batch
                for wtile, w2tile, rot in (
                    (wqs, wq2s, qt),
                    (wks, wk2s, kt),
                ):
                    pp = ps.tile([128, 512], F32, tag="big", bufs=2, name=f"pp{c}")
                    for k in range(DCH):
                        nc.tensor.matmul(
                            pp[:], wtile[:, k, :], xts[:, k, cs],
                            start=(k == 0), stop=(k == DCH - 1),
                        )
                    pp2 = ps.tile([128, 512], F32, tag="big", bufs=2, name=f"pp2{c}")
                    for k in range(DCH):
                        nc.tensor.matmul(
                            pp2[:], w2tile[:, k, :], xts[:, k, cs],
                            start=(k == 0), stop=(k == DCH - 1),
                        )
                    # rope: rot = raw*cos + raw2*sin (raw2 = signed-pair-swapped proj)
                    rtmp = work.tile([128, 512], BF, tag="ropetmp", bufs=2, name="rtmp")
                    nc.vector.tensor_mul(rot[:, cs], pp[:], coss[:, scol])
                    nc.vector.tensor_mul(rtmp[:], pp2[:], sins[:, scol])
                    nc.vector.tensor_add(rot[:, cs], rot[:, cs], rtmp[:])
                # v in natural orientation: out [s=128, e'=128] per s-tile
                for st in range(4):
                    t128 = c * 4 + st
                    vp = ps.tile([128, 128], F32, tag="big", bufs=2, name=f"vp{t128}")
                    for k in range(DCH):
                        nc.tensor.matmul(
                            vp[:], xts[:, k, ds(t128 * 128, 128)], wvs[:, k, :],
                            start=(k == 0), stop=(k == DCH - 1),
                        )
                    dst = vaug[:, t128, :].rearrange("p (g y) -> p g y", g=2)[:, :, 0:64]
                    src = vp[:].rearrange("p (g y) -> p g y", g=2)
                    nc.scalar.copy(dst, src)

            # ---- causal attention for one (batch, 512-wide q-chunk) ----
            def attn_chunk(b, j):
                ntk = 4 * (j + 1)
                pv = ps.tile([65, 1024], F32, tag="pv", bufs=1, name=f"pv{b}{j}")
                qc0 = b * S + j * 512
                for t in range(ntk):
                    off = max(0, 128 * (t - 4 * j))
                    w = 512 - off
                    sc = ps.tile([128, 2, 512], F32, tag="big", bufs=2, name="sc")
                    pt = work.tile([128, 2, 512], BF, tag="ptile", bufs=3, name="pt")
                    kc = b * S + t * 128
                    nc.tensor.matmul(
                        sc[:, 0, off:512], kt[0:64, ds(kc, 128)],
                        qt[0:64, ds(qc0 + off, w)], start=True, stop=True,
                    )
                    nc.tensor.matmul(
                        sc[:, 1, off:512], kt[64:128, ds(kc, 128)],
                        qt[64:128, ds(qc0 + off, w)], start=True, stop=True,
                    )
                    nc.scalar.activation(
                        pt[:, :, off:512], sc[:, :, off:512], EXPFN, scale=0.125,
                    )
                    if t >= 4 * j:  # diagonal: zero the sub-diagonal staircase
                        nc.vector.tensor_mul(
                            pt[:, :, off:off + 128], pt[:, :, off:off + 128],
                            mask2[:],
                        )
                    bt = b * NKT + t
                    nc.tensor.matmul(
                        pv[:, ds(off, w)], vaug[:, bt, 0:65], pt[:, 0, off:512],
                        start=(t == 0), stop=(t == ntk - 1),
                    )
                    nc.tensor.matmul(
                        pv[:, ds(512 + off, w)], vaug[:, bt, 65:130], pt[:, 1, off:512],
                        start=(t == 0), stop=(t == ntk - 1),
                    )
                # normalize by the denominator row (pv row 64)
                lbuf = work.tile([1, 1024], F32, tag="lbuf", bufs=2, name="lbuf")
                rbuf = work.tile([1, 1024], F32, tag="rbuf", bufs=2, name="rbuf")
                rb = work.tile([64, 1024], F32, tag="rb", bufs=2, name="rb")
                nc.vector.tensor_copy(lbuf[:], pv[64:65, :])
                nc.vector.reciprocal_approx_fast(rbuf[:], lbuf[:])
                nc.gpsimd.partition_broadcast(rb[:], rbuf[:], channels=64)
                oc = ds(b * S + j * 512, 512)
                nc.vector.tensor_mul(attnT[0:64, oc], pv[0:64, 0:512], rb[:, 0:512])
                nc.vector.tensor_mul(attnT[64:128, oc], pv[0:64, 512:1024], rb[:, 512:1024])

            # ---- output projection for one (batch, q-chunk) ----
            def oproj_chunk(b, j):
                oc = ds(b * S + j * 512, 512)
                ost = work.tile([128, 8, 512], BF, tag="ostage", bufs=2, name="ost")
                for e in range(8):
                    op = ps.tile([128, 512], F32, tag="op", bufs=2, name="op")
                    nc.tensor.matmul(
                        op[:], wos[:, ts(e, 128)], attnT[:, oc],
                        start=True, stop=True,
                    )
                    if e % 2 == 0:
                        nc.vector.tensor_copy(ost[:, e, :], op[:])
                    else:
                        nc.scalar.copy(ost[:, e, :], op[:])
                nc.sync.dma_start(
                    out=out_d[b].rearrange("(ec p) s -> p ec s", p=128)[:, :, ts(j, 512)],
                    in_=ost[:],
                )

            # interleave: projection of chunk i, attention on chunk i, and the
            # output projection of chunk i-1 (whose normalize finished during
            # attention of chunk i) are adjacent in program order so the PE
            # always has dense work while ACT/DVE chew the previous chunk
            chunks = [(b, j) for b in range(B) for j in range(SQJ)]
            for i, (b, j) in enumerate(chunks):
                proj_chunk(b * SQJ + j)
                attn_chunk(b, j)
                if i > 0:
                    oproj_chunk(*chunks[i - 1])
            oproj_chunk(*chunks[-1])

    nc.compile()
    _nc_cache[key] = nc
    return nc


def make_in_maps(x, token_positions, wq, wk, wv, wo):
    bf = ml_dtypes.bfloat16
    xT = np.ascontiguousarray(
        np.asarray(x, np.float32).transpose(2, 0, 1).reshape(D, BS)
    ).astype(bf)
    pos = np.asarray(token_positions, np.float64)
    inv_freq = THETA ** (-(2.0 * np.arange(HD // 2, dtype=np.float64) / HD))
    ang = pos[:, None] * inv_freq[None, :]          # [S, 32]
    cos = np.cos(ang).astype(np.float32)
    sin = np.sin(ang).astype(np.float32)
    p = np.arange(128)
    idx = (p % HD) // 2
    cosT = np.ascontiguousarray(cos[:, idx].T).astype(bf)             # [128, S]
    sinT = np.ascontiguousarray(sin[:, idx].T).astype(bf)

    wq = np.asarray(wq, np.float32)
    wk = np.asarray(wk, np.float32)
    wv = np.asarray(wv, np.float32)
    wo = np.asarray(wo, np.float32)

    def swap2(w):  # rows: even p -> -w[p+1], odd p -> +w[p-1]
        w2 = np.empty_like(w)
        w2[0::2] = -w[1::2]
        w2[1::2] = w[0::2]
        return w2

    in_maps = []
    for c in range(NCORES):
        rows = slice(c * EL, (c + 1) * EL)
        in_maps.append({
            "xT": xT,
            "cosT": cosT,
            "sinT": sinT,
            "wqT": np.ascontiguousarray(wq[rows, :].T).astype(bf),
            "wkT": np.ascontiguousarray(wk[rows, :].T).astype(bf),
            "wq2T": np.ascontiguousarray(swap2(wq[rows, :]).T).astype(bf),
            "wk2T": np.ascontiguousarray(swap2(wk[rows, :]).T).astype(bf),
            "wvT": np.ascontiguousarray(wv[rows, :].T).astype(bf),
            "woT": np.ascontiguousarray(wo[:, rows].T).astype(bf),
        })
    return in_maps


def unshard(results):
    acc = np.zeros((B, D, S), np.float32)
    for r in results:
        acc += np.asarray(r["out"], np.float32)
    return np.ascontiguousarray(acc.transpose(0, 2, 1))


def kernel(x, token_positions, wq, wk, wv, wo):
    from concourse.bass_utils import run_bass_kernel_spmd

    nc = build_nc(debug=False)
    in_maps = make_in_maps(x, token_positions, wq, wk, wv, wo)
    res = run_bass_kernel_spmd(nc, in_maps, core_ids=list(range(NCORES)))
    return unshard(res.results)


if __name__ == "__main__":
    # smoke test with random data
    rng = np.random.default_rng(0)
    x = rng.standard_normal((B, S, D), dtype=np.float32)
    tp = np.arange(S, dtype=np.int32)
    ws = [rng.standard_normal((D, D), dtype=np.float32) * 0.02 for _ in range(4)]
    out = kernel(x, tp, *ws)
    print(out.shape, out.dtype)
